# revision 4
# baseline (speedup 1.0000x reference)
"""Trainium2 Bass kernel for nn_Attention (B=4, N=2048, D=1024, H=16, DH=64).

Sharding: 8 cores = 4 batches x 2 head-halves (tensor-parallel heads).
Host: out[b] = partial[2b] + partial[2b+1] + bout.

v2 structure (single-pass-latency focused):
 - Priority DMA order: x chunk0 + pair0 weights first; fused multi-kt DMAs
   (11+8 transfers instead of 95).
 - Both heads' scores go into one 2-bank PSUM group [128, 2, 512]; ONE
   ACT exp instruction per (ic, jt) covers 1024 columns, amortizing the
   ~185ns ACT access latency. ACT runs exp only (no table thrash).
 - Attention inner loop software-pipelined: emit S(jt), exp(jt), fillers,
   then PV(jt-1) so the PE never waits on the ACT round-trip.
 - PE filler scheduling: v-projection inside pair0/ic0, next pair's q/k
   projection + rope inside the current pair's attention, output
   projection chunks inside pair3's attention (after each ic's norm).
 - Norm emission deferred one block (PE part waits a DVE chain).
 - RoPE combine written by DVE directly (no ACT copy).
 - PSUM: s0(2) s1(2) o0 o1 pq pk = 8 banks exactly.
"""

import sys

for _p in ("/opt/trn_rl_repo",):
    if _p not in sys.path:
        sys.path.insert(0, _p)

import numpy as np
import ml_dtypes

import concourse.bass as bass
import concourse.tile as tile
from concourse import library_config, mybir
from concourse.bass_utils import run_bass_kernel_spmd

BF16 = mybir.dt.bfloat16
F32 = mybir.dt.float32
NP_BF16 = ml_dtypes.bfloat16
EXP = mybir.ActivationFunctionType.Exp
IDENT = mybir.ActivationFunctionType.Identity

P = 128          # partitions
D = 1024         # model dim
INNER = 512      # per-core inner dim (8 heads * 64)
NH = 8           # heads per core
DH = 64          # head dim
KT = D // P      # 8 contraction tiles for projections
SCALE = DH ** -0.5


def build_nc(N=2048, IC=512, n_cores=8, debug_dumps=False,
             split_waits=True, repeat=1):
    NJT = N // P           # j tiles in attention / v row tiles
    NIC = N // IC          # i chunks (4)
    NCX = N // 512         # projection col chunks (4)

    nc = bass.Bass("TRN2", target_bir_lowering=False, debug=False,
                   num_devices=n_cores)

    xT = nc.dram_tensor("xT", [D, N], BF16, kind="ExternalInput").ap()
    wq = nc.dram_tensor("wq", [D, INNER], BF16, kind="ExternalInput").ap()
    wk = nc.dram_tensor("wk", [D, INNER], BF16, kind="ExternalInput").ap()
    wv = nc.dram_tensor("wv", [D, INNER], BF16, kind="ExternalInput").ap()
    wout = nc.dram_tensor("wout", [INNER, D], BF16, kind="ExternalInput").ap()
    cosT = nc.dram_tensor("cosT", [P, N], BF16, kind="ExternalInput").ap()
    sinT = nc.dram_tensor("sinT", [P, N], BF16, kind="ExternalInput").ap()
    rT = nc.dram_tensor("rT", [P, P], BF16, kind="ExternalInput").ap()
    out = nc.dram_tensor("out", [N, D], BF16, kind="ExternalOutput").ap()

    xT_r = xT.rearrange("(kt p) n -> p kt n", p=P)
    wq_r = wq.rearrange("(kt p) i -> p kt i", p=P)
    wk_r = wk.rearrange("(kt p) i -> p kt i", p=P)
    wv_r = wv.rearrange("(kt p) i -> p kt i", p=P)
    wout_r = wout.rearrange("(pt p) d -> p pt d", p=P)

    with tile.TileContext(nc) as tc, \
         nc.allow_low_precision(reason="bf16 softmax-reciprocal broadcast is "
                                "within the 2e-2 tolerance"):
        with tc.tile_pool(name="persist", bufs=1) as pp:
            wq_sb = pp.tile([P, KT, INNER], BF16, tag="wq")
            wk_sb = pp.tile([P, KT, INNER], BF16, tag="wk")
            wv_sb = pp.tile([P, KT, INNER], BF16, tag="wv")
            wout_sb = pp.tile([P, 4, D], BF16, tag="wout")
            cos_sb = pp.tile([P, N], BF16, tag="cos")
            sin_sb = pp.tile([P, N], BF16, tag="sin")
            rT_sb = pp.tile([P, P], BF16, tag="rT")
            xt_all = pp.tile([P, KT, N], BF16, tag="xt")
            qTr = pp.tile([P, 4, N], BF16, tag="qTr")
            kTr = pp.tile([P, 4, N], BF16, tag="kTr")
            vA = pp.tile([P, NJT, NH, DH + 1], BF16, tag="vA")
            outT = pp.tile([P, 4, N], BF16, tag="outT")

            # ---- priority-ordered input DMAs ----
            # ramp deps first: x chunk0, pair0 q/k weights, rope tables, wv;
            # then the rest. Issue rotates across three DGE queues
            # (SP/ACT/DVE) so transfers run in parallel.
            dge = [nc.sync]
            dge_i = [0]

            def dma_in(out_ap, in_ap):
                eng = dge[dge_i[0] % len(dge)]
                dge_i[0] += 1
                eng.dma_start(out=out_ap, in_=in_ap)

            def dma_x(xc):
                xsl = slice(xc * 512, (xc + 1) * 512)
                dma_in(xt_all[:, :, xsl], xT_r[:, :, xsl])

            def dma_w_pair(w_sb, w_r, pair):
                msl = slice(pair * P, (pair + 1) * P)
                dma_in(w_sb[:, :, msl], w_r[:, :, msl])

            # x chunk0 in halves so the split first proj units start sooner
            dma_in(xt_all[:, :, 0:256], xT_r[:, :, 0:256])
            dma_w_pair(wq_sb, wq_r, 0)
            dma_in(xt_all[:, :, 256:512], xT_r[:, :, 256:512])
            dma_in(cos_sb, cosT)
            dma_in(sin_sb, sinT)
            dma_w_pair(wk_sb, wk_r, 0)
            dma_in(rT_sb, rT)
            dma_in(wv_sb, wv_r)
            dma_x(1)
            dma_x(2)
            dma_x(3)
            for pr in range(1, 4):
                dma_w_pair(wq_sb, wq_r, pr)
                dma_w_pair(wk_sb, wk_r, pr)
            dma_in(wout_sb, wout_r)

            # ones column for fused row-sum in PV matmul
            nc.scalar.activation(
                vA[:, :, :, DH:DH + 1],
                cos_sb[:, 0:NJT * NH].rearrange("p (a b) -> p a b", a=NJT)[:, :, :, None],
                IDENT, bias=1.0, scale=0.0)
            # bf16 ones row: stationary of the K=1 reciprocal-broadcast matmul
            ones1 = pp.tile([1, DH], BF16, tag="ones1")
            nc.scalar.activation(ones1, cos_sb[0:1, 0:DH], IDENT,
                                 bias=1.0, scale=0.0)

            for _rep in range(repeat):
                with tc.tile_pool(name="ps", bufs=1, space="PSUM") as ps, \
                     tc.tile_pool(name="sb", bufs=1) as sbp:

                    if _rep > 0:
                        for xc in range(NCX):
                            dma_x(xc)

                    ptag_state = [0]

                    def ptag():
                        ptag_state[0] ^= 1
                        return "pq" if ptag_state[0] else "pk"

                    # ---------- unit emitters ----------
                    def proj_unit(pair, cx, which, split=False):
                        """q/k projection chunk: 8 matmuls + DVE copy into a
                        transient qkt tile, for rope to consume. split=True
                        runs in two column halves so the first half only
                        needs half of the x chunk (faster ramp)."""
                        msl = slice(pair * P, (pair + 1) * P)
                        w_sb = wq_sb if which == "q" else wk_sb
                        ps_p = ps.tile([P, 512], F32, tag=ptag(), bufs=1,
                                       name=f"ps_{which}{pair}_{cx}")
                        halves = ((slice(0, 256), slice(256, 512))
                                  if split else (slice(0, 512),))
                        for hs in halves:
                            nsl = slice(cx * 512 + hs.start, cx * 512 + hs.stop)
                            for kt in range(KT):
                                nc.tensor.matmul(ps_p[:, hs], w_sb[:, kt, msl],
                                                 xt_all[:, kt, nsl],
                                                 start=(kt == 0), stop=(kt == KT - 1))
                        t = sbp.tile([P, 512], BF16, tag="qkt", bufs=4,
                                     name=f"qk_{which}{pair}_{cx}")
                        nc.vector.tensor_copy(t, ps_p)
                        return t

                    def rope_unit(pair, cx, src_t, dst):
                        """qTr/kTr[:, pair, chunk] = src*cos + (R@src)*sin."""
                        sl = slice(cx * 512, (cx + 1) * 512)
                        ps_r = ps.tile([P, 512], F32, tag=ptag(), bufs=1,
                                       name=f"ps_rot{pair}_{cx}")
                        nc.tensor.matmul(ps_r, rT_sb, src_t,
                                         start=True, stop=True)
                        t1 = sbp.tile([P, 512], BF16, tag="t1", bufs=2)
                        nc.vector.tensor_mul(t1, src_t, cos_sb[:, sl])
                        t2 = sbp.tile([P, 512], BF16, tag="t2", bufs=2)
                        nc.vector.tensor_mul(t2, ps_r, sin_sb[:, sl])
                        nc.vector.tensor_add(dst[:, pair, sl], t1, t2)

                    def v_unit(jt):
                        """v rows for j-tile jt: 8 matmuls + DVE copy to vA."""
                        r2sl = slice(jt * P, (jt + 1) * P)
                        ps_v = ps.tile([P, INNER], F32, tag=ptag(), bufs=1,
                                       name=f"ps_v{jt}")
                        for kt in range(KT):
                            nc.tensor.matmul(ps_v, xt_all[:, kt, r2sl],
                                             wv_sb[:, kt, :],
                                             start=(kt == 0), stop=(kt == KT - 1))
                        nc.vector.tensor_copy(
                            vA[:, jt, :, 0:DH],
                            ps_v.rearrange("p (h d) -> p h d", h=NH))

                    def outproj_unit(rtile, ncx2, tag=None, on_act=False):
                        rsl = slice(rtile * P, (rtile + 1) * P)
                        nsl = slice(ncx2 * 512, (ncx2 + 1) * 512)
                        ps_f = ps.tile([P, 512], F32, tag=tag or ptag(), bufs=1,
                                       name=f"ps_f{rtile}_{ncx2}")
                        for p4 in range(4):
                            nc.tensor.matmul(ps_f, outT[:, p4, rsl],
                                             wout_sb[:, p4, nsl],
                                             start=(p4 == 0), stop=(p4 == 3))
                        fin = sbp.tile([P, 512], BF16, tag="fin", bufs=3)
                        if on_act:
                            nc.scalar.copy(fin, ps_f)
                        else:
                            nc.vector.tensor_copy(fin, ps_f)
                        nc.sync.dma_start(out=out[rsl, nsl], in_=fin)

                    def norm_emit(pend):
                        """PE broadcast of 1/rowsum + DVE scale into outT."""
                        pair, ic, ocs, lrecs = pend
                        isl = slice(ic * IC, (ic + 1) * IC)
                        for lc in range(2):
                            prow = slice(lc * DH, (lc + 1) * DH)
                            ps_l = ps.tile([DH, IC], F32, tag=ptag(), bufs=1,
                                           name=f"ps_l{pair}_{ic}_{lc}")
                            nc.tensor.matmul(ps_l, ones1, lrecs[lc],
                                             start=True, stop=True)
                            nc.vector.tensor_mul(outT[prow, pair, isl],
                                                 ocs[lc], ps_l)

                    # ---------- filler schedules ----------
                    def pair_sched(pair):
                        s = {}

                        def add(ic, jt, fn):
                            s.setdefault((ic, jt), []).append(fn)

                        def sched_next_pair(pp_, slots):
                            """Interleave pair pp_'s full proj+rope as filler
                            at the given (ic, jt) slots (needs 16)."""
                            src = {}
                            units = []
                            for cx in range(NCX):
                                units.append(("pj", "q", cx))
                                units.append(("pj", "k", cx))
                                units.append(("rp", "q", cx))
                                units.append(("rp", "k", cx))
                            for (ic, jt), u in zip(slots, units):
                                kind, which, cx = u
                                if kind == "pj":
                                    add(ic, jt,
                                        lambda which=which, cx=cx:
                                        src.setdefault((which, cx),
                                                       proj_unit(pp_, cx, which)))
                                else:
                                    dst = qTr if which == "q" else kTr
                                    add(ic, jt,
                                        lambda which=which, cx=cx, dst=dst:
                                        rope_unit(pp_, cx, src[(which, cx)], dst))

                        if pair == 0:
                            # v-projection rides in ic0 (needed by first PV
                            # accumulation); remaining k chunks + their rope
                            # land just before the S tiles that read them.
                            for jt in range(NJT):
                                add(0, jt, lambda jt=jt: v_unit(jt))
                            ksrc = {}
                            add(0, 0, lambda: ksrc.setdefault(1, proj_unit(0, 1, "k")))
                            add(0, 1, lambda: rope_unit(0, 1, ksrc[1], kTr))
                            add(0, 4, lambda: ksrc.setdefault(2, proj_unit(0, 2, "k")))
                            add(0, 5, lambda: rope_unit(0, 2, ksrc[2], kTr))
                            add(0, 8, lambda: ksrc.setdefault(3, proj_unit(0, 3, "k")))
                            add(0, 9, lambda: rope_unit(0, 3, ksrc[3], kTr))
                            qsrc = {}
                            add(0, 11, lambda: qsrc.setdefault(1, proj_unit(0, 1, "q")))
                            add(0, 12, lambda: rope_unit(0, 1, qsrc[1], qTr))
                            add(1, 3, lambda: qsrc.setdefault(2, proj_unit(0, 2, "q")))
                            add(1, 4, lambda: rope_unit(0, 2, qsrc[2], qTr))
                            add(2, 3, lambda: qsrc.setdefault(3, proj_unit(0, 3, "q")))
                            add(2, 4, lambda: rope_unit(0, 3, qsrc[3], qTr))
                            # pair 1's projections also ride in pair 0
                            sched_next_pair(1, [(1, jt) for jt in (5, 7, 9, 11, 13, 15)]
                                            + [(2, jt) for jt in (5, 7, 9, 11, 13, 15)]
                                            + [(3, jt) for jt in (3, 5, 7, 9)])
                        elif pair in (1, 2):
                            sched_next_pair(pair + 1,
                                            [(ic, jt) for ic in range(NIC)
                                             for jt in (3, 7, 11, 15)])
                        else:  # pair 3: output projection after each norm
                            for ic in range(1, NIC):
                                units = [(rt, cx2) for rt in range(4 * (ic - 1),
                                                                   4 * ic)
                                         for cx2 in range(2)]
                                slots = [5, 7, 9, 11, 13, 13, 15, 15]
                                for jt, (rt, cx2) in zip(slots, units):
                                    add(ic, jt,
                                        lambda rt=rt, cx2=cx2: outproj_unit(rt, cx2))
                        return s

                    # ---------- attention ----------
                    # The PV queue crosses block (ic/pair) boundaries so the
                    # last PVs of a block drain under the next block's S/exp
                    # groups instead of stalling on the final exps. Each
                    # entry carries its own accumulator; the DVE norm prep
                    # for a block is emitted right after its PV(15).
                    pending_norm = [None]
                    pvq = []

                    def emit_pv(pt_t, jt, ps_o, pair, ic):
                        for lc in range(2):
                            h = pair * 2 + lc
                            nc.tensor.matmul(
                                ps_o[lc], vA[:, jt, h, :],
                                pt_t[:, lc, :],
                                start=(jt == 0), stop=(jt == NJT - 1))
                        if jt == NJT - 1:
                            norm_prep(ps_o, pair, ic)

                    def norm_prep(ps_o, pr, ic):
                        ocs, lrecs = [], []
                        for lc in range(2):
                            lrec = sbp.tile([1, IC], BF16, tag=f"lr{lc}",
                                            bufs=2, name=f"lr{pr}_{ic}_{lc}")
                            nc.vector.reciprocal(lrec, ps_o[lc][DH:DH + 1, :])
                            oc = sbp.tile([DH, IC], F32, tag=f"oc{lc}",
                                          bufs=2, name=f"oc{pr}_{ic}_{lc}")
                            nc.vector.tensor_copy(oc, ps_o[lc][0:DH, :])
                            ocs.append(oc)
                            lrecs.append(lrec)
                        pending_norm[0] = (pr, ic, ocs, lrecs)

                    def attention_pair(pair):
                        sched = pair_sched(pair)
                        for ic in range(NIC):
                            isl = slice(ic * IC, (ic + 1) * IC)
                            ps_o = [ps.tile([DH + 1, IC], F32, tag=f"o{lc}",
                                            bufs=1, name=f"ps_o{pair}_{ic}_{lc}")
                                    for lc in range(2)]

                            for jt in range(NJT):
                                jsl = slice(jt * P, (jt + 1) * P)
                                grp = ps.tile([P, 2, IC], F32,
                                              tag=f"s{jt % 2}", bufs=1,
                                              name=f"ps_s{pair}_{ic}_{jt}")
                                for lc in range(2):
                                    prow = slice(lc * DH, (lc + 1) * DH)
                                    nc.tensor.matmul(grp[:, lc, :],
                                                     kTr[prow, pair, jsl],
                                                     qTr[prow, pair, isl],
                                                     start=True, stop=True)
                                pt_t = sbp.tile([P, 2, IC], BF16, tag="pt",
                                                bufs=4, name=f"pt{pair}_{ic}_{jt}")
                                nc.scalar.activation(pt_t, grp, EXP, scale=SCALE)
                                for fn in sched.get((ic, jt), ()):
                                    fn()
                                if jt == 3 and pending_norm[0] is not None:
                                    norm_emit(pending_norm[0])
                                    pending_norm[0] = None
                                pvq.append((pt_t, jt, ps_o, pair, ic))
                                if len(pvq) > 2:
                                    emit_pv(*pvq.pop(0))

                    # ---------- main emission ----------
                    # ramp: pair0 chunk-0 projections + rope
                    q0 = proj_unit(0, 0, "q", split=(_rep == 0))
                    k0 = proj_unit(0, 0, "k", split=(_rep == 0))
                    rope_unit(0, 0, q0, qTr)
                    rope_unit(0, 0, k0, kTr)

                    for pair in range(4):
                        attention_pair(pair)
                    for pv in pvq:
                        emit_pv(*pv)
                    pvq.clear()

                    # tail: last norm + remaining outproj rotating over four
                    # PSUM tags (attention banks are free now). Results
                    # collect into one fat tile and leave in a single DMA so
                    # the drain isn't gated by eight serialized transfers.
                    norm_emit(pending_norm[0])
                    pending_norm[0] = None
                    tail_tags = ["pq", "pk", "o0", "o1"]
                    fat = sbp.tile([P, 4, D], BF16, tag="fat", bufs=1)
                    ti = 0
                    for rt in range(12, 16):
                        for cx2 in range(2):
                            rsl = slice(rt * P, (rt + 1) * P)
                            nsl = slice(cx2 * 512, (cx2 + 1) * 512)
                            ps_f = ps.tile([P, 512], F32,
                                           tag=tail_tags[ti % 4], bufs=1,
                                           name=f"ps_ft{rt}_{cx2}")
                            for p4 in range(4):
                                nc.tensor.matmul(ps_f, outT[:, p4, rsl],
                                                 wout_sb[:, p4, nsl],
                                                 start=(p4 == 0), stop=(p4 == 3))
                            if ti % 2 == 1:
                                nc.scalar.copy(fat[:, rt - 12, nsl], ps_f)
                            else:
                                nc.vector.tensor_copy(fat[:, rt - 12, nsl], ps_f)
                            ti += 1
                    nc.sync.dma_start(
                        out=out[12 * P:16 * P, :].rearrange(
                            "(rt p) d -> p rt d", p=P),
                        in_=fat)

                    if debug_dumps:
                        for nm, t in (("d_qT", qTr), ("d_kT", kTr),
                                      ("d_outT", outT)):
                            dmp = nc.dram_tensor(nm, [P, 4, N], BF16,
                                                 kind="ExternalOutput").ap()
                            nc.sync.dma_start(out=dmp, in_=t)
                        dv = nc.dram_tensor("d_vA", [P, NJT, NH, DH + 1], BF16,
                                            kind="ExternalOutput").ap()
                        nc.sync.dma_start(out=dv, in_=vA)

    if split_waits:
        _split_sync_waits(nc)
    return nc


_SYNC_EXEMPT = {"InstEventSemaphore", "InstAllEngineBarrier",
                "InstNoOp", "InstUnconditionalBranch", "InstCompareAndBranch",
                "InstHalt", "InstBranchHint"}


def _split_sync_waits(nc, cap_total=2):
    """Compact TPB instruction encodings only fit ~2 sync commands
    (waits+updates). Hoist excess waits onto same-engine InstNoOps inserted
    immediately before (waits strictly earlier in the same stream)."""
    for fn in nc.m.functions:
        for bb in fn.blocks:
            il = bb.instructions
            n = 0
            while n < len(il):
                i = il[n]
                nm = type(i).__name__
                si = i.sync_info
                if nm in _SYNC_EXEMPT or si is None:
                    n += 1
                    continue
                waits = list(si.on_wait or [])
                upds = list(si.on_update or [])
                allowed = 0 if len(upds) >= 2 else 1
                if len(waits) <= allowed:
                    n += 1
                    continue
                keep = waits[-allowed:] if allowed else []
                excess = waits[:len(waits) - allowed]
                pos = n
                while excess:
                    chunk, excess = excess[:1], excess[1:]
                    nop = mybir.InstNoOp(
                        name=nc.get_next_instruction_name(),
                        engine=i.engine,
                        bass_nofuse=True,
                        sync_info=mybir.SyncInfo(on_wait=chunk, on_update=[]),
                    )
                    il.insert(pos, nop)
                    pos += 1
                si.on_wait = keep
                n = pos + 1


def _rot_matrix_T():
    """R^T such that (R @ tT) = rotate_half(t)^T in [h*64+d, n] layout."""
    r64 = np.zeros((DH, DH), dtype=np.float32)
    for dp in range(32):
        r64[dp, dp + 32] = -1.0
        r64[dp + 32, dp] = 1.0
    r = np.zeros((P, P), dtype=np.float32)
    r[:DH, :DH] = r64
    r[DH:, DH:] = r64
    return np.ascontiguousarray(r.T.astype(NP_BF16))


def make_in_maps(x, rotary_emb, Wq, Wkv, Wout, n_cores=8):
    B, N, Dm = x.shape
    rT = _rot_matrix_T()
    cosT = np.tile(np.cos(rotary_emb.astype(np.float64)).T, (2, 1)).astype(NP_BF16)
    sinT = np.tile(np.sin(rotary_emb.astype(np.float64)).T, (2, 1)).astype(NP_BF16)
    cosT = np.ascontiguousarray(cosT)
    sinT = np.ascontiguousarray(sinT)
    wk_full = Wkv[:, :Dm]
    wv_full = Wkv[:, Dm:]
    in_maps = []
    for c in range(n_cores):
        b, hh = c // 2, c % 2
        sl = slice(hh * INNER, (hh + 1) * INNER)
        in_maps.append({
            "xT": np.ascontiguousarray(x[b].T).astype(NP_BF16),
            "wq": np.ascontiguousarray(Wq[:, sl]).astype(NP_BF16),
            "wk": np.ascontiguousarray(wk_full[:, sl]).astype(NP_BF16),
            "wv": np.ascontiguousarray(wv_full[:, sl]).astype(NP_BF16),
            "wout": np.ascontiguousarray(Wout[sl, :]).astype(NP_BF16),
            "cosT": cosT,
            "sinT": sinT,
            "rT": rT,
        })
    return in_maps


_NC_CACHE = {}


def kernel(x, rotary_emb, Wq, Wkv, Wout, bout, _trace=False):
    B, N, Dm = x.shape
    if "nc" not in _NC_CACHE:
        _NC_CACHE["nc"] = build_nc(N=N)
    nc = _NC_CACHE["nc"]
    in_maps = make_in_maps(x, rotary_emb, Wq, Wkv, Wout)
    res = run_bass_kernel_spmd(nc, in_maps, core_ids=list(range(8)),
                               trace=_trace)
    outs = [res.results[c]["out"] for c in range(8)]
    full = np.empty((B, N, Dm), dtype=np.float32)
    for b in range(B):
        full[b] = (outs[2 * b].astype(np.float32)
                   + outs[2 * b + 1].astype(np.float32)
                   + bout[None, :].astype(np.float32))
    if _trace:
        return full, res
    return full
